# revision 10
# baseline (speedup 1.0000x reference)
"""DiKT (DKVMN-style knowledge tracing) Trainium2 kernel.

Self-contained: builds a Bass/Tile program, shards batch over 8 NeuronCores
(pure data parallel, 16 batch rows per core), runs via run_bass_kernel_spmd.

Algorithm per core (B_loc=16, V=128, C=64, S=128 steps):
  Both value memories (right/wrong) live as ONE SBUF tensor m[v=128, col=2048]
  with col = r*64 + c, r = mem*16 + b.  Per step:
     m' = m * (1 - e x w) + a x w
  The rank-1 outer products are built on the TensorEngine using a
  block-diagonal (negated) w matrix:  wdiag[r, col] = -w[r,c] * (r block),
     E' = e_t^T . wdiag   (= -e x w),   S = 1 + E'   (ScalarE bias)
     A  = (-a_t)^T . wdiag (= +a x w)
  DVE:  u = m * S ;  m = u + A   (fp16 SBUF, 2x mode)

All per-step e, a, w are precomputed up front from embedding gathers
(indirect DMA) + small matmuls; wdiag for all steps is staged in DRAM and
streamed during the loop.
"""

import numpy as np

import concourse.mybir as mybir
from concourse import bass, bacc, tile
from concourse.bass_utils import run_bass_kernel_spmd

F16 = mybir.dt.float16
F32 = mybir.dt.float32
I32 = mybir.dt.int32
ALU = mybir.AluOpType
ACT = mybir.ActivationFunctionType
AX = mybir.AxisListType

# model dims
KD = 128      # KEY_DIM
VD = 128      # VALUE_DIM
SD = 128      # SUMMARY_DIM
Q = 10000     # QUESTION_NUM
C = 64        # CONCEPT_NUM
B = 128       # full batch
S = 128       # seq len
NCORE = 8
BL = B // NCORE          # 16 batch rows per core
NR = 2 * BL              # 32 rows per step (right+wrong)
COLS = NR * C            # 2048 memory columns per core
NG = (S * NR) // 128     # 32 gather chunks of 128 rows
WD_STEP = NR * COLS // NR  # elements per (step,row) = 2048
WD_T = NR * COLS // NR * NR  # 65536 elements per step block (32*2048)

DEBUG = False


def _build_program():
    nc = bacc.Bacc(trn_type="TRN2", target_bir_lowering=False, num_devices=NCORE,
                   num_swdge_queues=4)

    # ---- DRAM inputs ----
    i_emb = nc.dram_tensor("i_emb", [2 * Q + 1, VD], F32, kind="ExternalInput")
    q_emb = nc.dram_tensor("q_emb", [Q + 1, KD], F32, kind="ExternalInput")
    idx_i = nc.dram_tensor("idx_i", [128, NG], I32, kind="ExternalInput")
    idx_q = nc.dram_tensor("idx_q", [128, NG], I32, kind="ExternalInput")
    idx_t = nc.dram_tensor("idx_t", [BL, 1], I32, kind="ExternalInput")

    erase_Wt = nc.dram_tensor("erase_Wt", [VD, VD], F16, kind="ExternalInput")
    add_Wt = nc.dram_tensor("add_Wt", [VD, VD], F16, kind="ExternalInput")
    key_Wt = nc.dram_tensor("key_Wt", [KD, C], F16, kind="ExternalInput")
    erase_b_row = nc.dram_tensor("erase_b_row", [1, VD], F16, kind="ExternalInput")
    add_b_row = nc.dram_tensor("add_b_row", [1, VD], F16, kind="ExternalInput")
    rsum_Wt0 = nc.dram_tensor("rsum_Wt0", [VD, SD], F16, kind="ExternalInput")
    rsum_Wt1 = nc.dram_tensor("rsum_Wt1", [KD, SD], F16, kind="ExternalInput")
    wsum_Wt0 = nc.dram_tensor("wsum_Wt0", [VD, SD], F16, kind="ExternalInput")
    wsum_Wt1 = nc.dram_tensor("wsum_Wt1", [KD, SD], F16, kind="ExternalInput")
    rsum_b_col = nc.dram_tensor("rsum_b_col", [SD, 1], F32, kind="ExternalInput")
    wsum_b_col = nc.dram_tensor("wsum_b_col", [SD, 1], F32, kind="ExternalInput")
    succ_Wt = nc.dram_tensor("succ_Wt", [SD, 1], F16, kind="ExternalInput")
    fail_Wt = nc.dram_tensor("fail_Wt", [SD, 1], F16, kind="ExternalInput")
    diff_Wt = nc.dram_tensor("diff_Wt", [KD, 1], F16, kind="ExternalInput")
    succ_b = nc.dram_tensor("succ_b", [1, 1], F32, kind="ExternalInput")
    fail_b = nc.dram_tensor("fail_b", [1, 1], F32, kind="ExternalInput")
    diff_b = nc.dram_tensor("diff_b", [1, 1], F32, kind="ExternalInput")
    rmem0 = nc.dram_tensor("rmem0", [VD, C], F16, kind="ExternalInput")
    wmem0 = nc.dram_tensor("wmem0", [VD, C], F16, kind="ExternalInput")
    ones_row = nc.dram_tensor("ones_row", [1, 128], F16, kind="ExternalInput")
    ones_col32 = nc.dram_tensor("ones_col32", [128, 1], F32, kind="ExternalInput")
    id128 = nc.dram_tensor("id128", [128, 128], F16, kind="ExternalInput")
    right_full = nc.dram_tensor("right_full", [B, S], I32, kind="ExternalInput")
    wrong_full = nc.dram_tensor("wrong_full", [B, S], I32, kind="ExternalInput")

    out_d = nc.dram_tensor("out", [BL, 1], F32, kind="ExternalOutput")
    if DEBUG:
        dbg_m = nc.dram_tensor("dbg_m", [VD, COLS], F16, kind="ExternalOutput")
        dbg_e = nc.dram_tensor("dbg_e", [128, S], F16, kind="ExternalOutput")
        dbg_na = nc.dram_tensor("dbg_na", [128, S], F16, kind="ExternalOutput")
        dbg_rr = nc.dram_tensor("dbg_rr", [VD, NR], F32, kind="ExternalOutput")
        dbg_wd = nc.dram_tensor("dbg_wd", [NR, COLS], F16, kind="ExternalOutput")

    # wdiag for every step, flat fp16: element (t, r, col) at t*65536 + r*2048 + col
    wd_dram = nc.dram_tensor("wd_dram", [S * NR * COLS], F16)

    # ---- persistent SBUF ----
    sb = lambda name, shape, dt: nc.alloc_sbuf_tensor(name, shape, dt)
    m_sb = sb("m_sb", [VD, COLS], F16)
    vecT = sb("vecT", [128, NG * 128], F16)   # i_emb rows, transposed, fp16
    qT = sb("qT", [128, NG * 128], F16)
    e_all = sb("e_all", [128, NG * 128], F16)  # sigmoid(erase)
    na_all = sb("na_all", [128, NG * 128], F16)  # -tanh(add)
    w_eWt = sb("w_eWt", [VD, VD], F16)
    w_aWt = sb("w_aWt", [VD, VD], F16)
    w_kWt = sb("w_kWt", [KD, C], F16)
    w_eb = sb("w_eb", [1, VD], F16)
    w_ab = sb("w_ab", [1, VD], F16)
    w_ones = sb("w_ones", [1, 128], F16)
    w_ones_c32 = sb("w_ones_c32", [128, 1], F32)
    w_id = sb("w_id", [128, 128], F16)
    idx_i_sb = sb("idx_i_sb", [128, NG], I32)
    idx_q_sb = sb("idx_q_sb", [128, NG], I32)
    idx_t_sb = sb("idx_t_sb", [BL, 1], I32)
    w_rs0 = sb("w_rs0", [VD, SD], F16)
    w_rs1 = sb("w_rs1", [KD, SD], F16)
    w_ws0 = sb("w_ws0", [VD, SD], F16)
    w_ws1 = sb("w_ws1", [KD, SD], F16)
    w_rsb = sb("w_rsb", [SD, 1], F32)
    w_wsb = sb("w_wsb", [SD, 1], F32)
    w_succ = sb("w_succ", [SD, 1], F16)
    w_fail = sb("w_fail", [SD, 1], F16)
    w_diff = sb("w_diff", [KD, 1], F16)
    w_sb_b = sb("w_sb_b", [1, 3], F32)  # succ_b, fail_b, diff_b columns 0..2
    zeros2k = sb("zeros2k", [128, COLS], F16)

    with tile.TileContext(nc) as tc:
        with tc.tile_pool(name="sbp", bufs=3) as sbp:
            # ---------- load constants ----------
            for dst, src in [
                (w_eWt, erase_Wt), (w_aWt, add_Wt), (w_kWt, key_Wt),
                (w_eb, erase_b_row), (w_ab, add_b_row), (w_ones, ones_row),
                (w_ones_c32, ones_col32), (w_id, id128),
                (idx_i_sb, idx_i), (idx_q_sb, idx_q), (idx_t_sb, idx_t),
                (w_rs0, rsum_Wt0), (w_rs1, rsum_Wt1),
                (w_ws0, wsum_Wt0), (w_ws1, wsum_Wt1),
                (w_rsb, rsum_b_col), (w_wsb, wsum_b_col),
                (w_succ, succ_Wt), (w_fail, fail_Wt), (w_diff, diff_Wt),
            ]:
                nc.sync.dma_start(out=dst[:, :], in_=src[:, :])
            nc.sync.dma_start(out=w_sb_b[:, 0:1], in_=succ_b[:, :])
            nc.sync.dma_start(out=w_sb_b[:, 1:2], in_=fail_b[:, :])
            nc.sync.dma_start(out=w_sb_b[:, 2:3], in_=diff_b[:, :])

            # zero-fill wd_dram (16 MiB fp16) from a zeroed sbuf tile
            nc.gpsimd.memset(zeros2k[:, :], 0.0)
            for g in range(NG):
                nc.scalar.dma_start(
                    out=bass.AP(wd_dram, g * 4 * WD_T, [[2048, 128], [1, 2048]]),
                    in_=zeros2k[:, :],
                )

            # init m: broadcast mem inits over the 16 batch blocks
            rmem_t = sbp.tile([VD, C], F16, tag="memi")
            nc.sync.dma_start(out=rmem_t[:, :], in_=rmem0[:, :])
            wmem_t = sbp.tile([VD, C], F16, tag="memi2")
            nc.sync.dma_start(out=wmem_t[:, :], in_=wmem0[:, :])
            for r in range(NR):
                srct = rmem_t if r < BL else wmem_t
                nc.vector.tensor_copy(m_sb[:, r * C:(r + 1) * C], srct[:, :])

            # ---------- gathers + transposes (xbar DMA transpose) ----------
            for g in range(NG):
                lo = g * 128
                # i-table chunk
                gi32 = sbp.tile([128, VD], F32, tag="gi32")
                nc.gpsimd.indirect_dma_start(
                    out=gi32[:, :], out_offset=None,
                    in_=i_emb[:, :],
                    in_offset=bass.IndirectOffsetOnAxis(
                        ap=idx_i_sb[:, g:g + 1], axis=0),
                )
                gi16 = sbp.tile([128, VD], F16, tag="gi16")
                nc.vector.tensor_copy(gi16[:, :], gi32[:, :])
                nc.sync.dma_start(out=vecT[:, lo:lo + 128], in_=gi16[:, :],
                                  transpose=True)
                # q-table chunk
                gq32 = sbp.tile([128, KD], F32, tag="gq32")
                nc.gpsimd.indirect_dma_start(
                    out=gq32[:, :], out_offset=None,
                    in_=q_emb[:, :],
                    in_offset=bass.IndirectOffsetOnAxis(
                        ap=idx_q_sb[:, g:g + 1], axis=0),
                )
                gq16 = sbp.tile([128, KD], F16, tag="gq16")
                nc.vector.tensor_copy(gq16[:, :], gq32[:, :])
                nc.scalar.dma_start(out=qT[:, lo:lo + 128], in_=gq16[:, :],
                                    transpose=True)

            # ---------- e/a/w precompute ----------
            # grouped by ACT table set: all Sigmoid, then all Tanh, then Exp
            with tc.tile_pool(name="psz", bufs=2, space="PSUM") as psp:
                for gg in range(NG // 4):
                    # e = sigmoid(vec @ erase_W.T + erase_b), batched 4 chunks
                    eps = psp.tile([128, 512], F32, tag="eps")
                    for k in range(4):
                        lo = (4 * gg + k) * 128
                        nc.tensor.matmul(eps[:, 128 * k:128 * (k + 1)],
                                         vecT[:, lo:lo + 128], w_eWt[:, :],
                                         start=True, stop=False)
                        nc.tensor.matmul(eps[:, 128 * k:128 * (k + 1)],
                                         w_ones[:, :], w_eb[:, :],
                                         start=False, stop=True)
                    nc.scalar.activation(e_all[:, 512 * gg:512 * (gg + 1)],
                                         eps[:, :], ACT.Sigmoid)
                for gg in range(NG // 4):
                    # na = -tanh(vec @ add_W.T + add_b), batched 4 chunks
                    aps = psp.tile([128, 512], F32, tag="aps")
                    for k in range(4):
                        lo = (4 * gg + k) * 128
                        nc.tensor.matmul(aps[:, 128 * k:128 * (k + 1)],
                                         vecT[:, lo:lo + 128], w_aWt[:, :],
                                         start=True, stop=False)
                        nc.tensor.matmul(aps[:, 128 * k:128 * (k + 1)],
                                         w_ones[:, :], w_ab[:, :],
                                         start=False, stop=True)
                    nc.scalar.activation(na_all[:, 512 * gg:512 * (gg + 1)],
                                         aps[:, :], ACT.Tanh, scale=-1.0)
                for g in range(NG):
                    lo = g * 128
                    # w = softmax(qv @ key_W.T)  (|z| <~ 4, exp is fp32-safe
                    # without max subtraction); store NEGATED into wdiag dram
                    zps = psp.tile([128, C], F32, tag="zps")
                    nc.tensor.matmul(zps[:, :], qT[:, lo:lo + 128], w_kWt[:, :],
                                     start=True, stop=True)
                    wex = sbp.tile([128, C], F32, tag="wex")
                    nc.scalar.activation(wex[:, :], zps[:, :], ACT.Exp)
                    sm = sbp.tile([128, 1], F32, tag="sm")
                    nc.vector.tensor_reduce(sm[:, :], wex[:, :], AX.X, ALU.add)
                    rc = sbp.tile([128, 1], F32, tag="rc")
                    nc.vector.reciprocal(rc[:, :], sm[:, :])
                    wng = sbp.tile([128, C], F16, tag="wng")
                    nc.vector.tensor_scalar(wng[:, :], wex[:, :], rc[:, :], -1.0,
                                            ALU.mult, ALU.mult)
                    # scatter the 4 steps' diagonal blocks
                    nc.sync.dma_start(
                        out=bass.AP(wd_dram, 4 * g * WD_T,
                                    [[WD_T, 4], [COLS + C, NR], [1, C]]),
                        in_=wng[:, :],
                    )

            # ---------- the recurrence ----------
            with tc.tile_pool(name="psl", bufs=1, space="PSUM") as psl, \
                 tc.tile_pool(name="wdp", bufs=2) as wdp, \
                 tc.tile_pool(name="sul", bufs=2) as sul:
                for t in range(S):
                    g, s = divmod(t, 4)
                    if s == 0:
                        wd4 = wdp.tile([128, COLS], F16, tag="wd4")
                        nc.sync.dma_start(
                            out=wd4[:, :],
                            in_=bass.AP(wd_dram, 4 * g * WD_T,
                                        [[2048, 128], [1, 2048]]),
                        )
                    p0 = 32 * s
                    H = COLS // 2
                    eL = e_all[p0:p0 + 32, g * 128:(g + 1) * 128]
                    aL = na_all[p0:p0 + 32, g * 128:(g + 1) * 128]
                    wdr = wd4[p0:p0 + 32, :]
                    # half 0 (cols [0,H)): ACT-evacuated path + POOL add
                    Eh0 = psl.tile([128, H], F32, tag="Eh0")
                    Ah0 = psl.tile([128, H], F32, tag="Ah0")
                    Eh1 = psl.tile([128, H], F32, tag="Eh1")
                    Ah1 = psl.tile([128, H], F32, tag="Ah1")
                    for hk in range(2):
                        c0, c1 = 512 * hk, 512 * (hk + 1)
                        nc.tensor.matmul(Eh0[:, c0:c1], eL, wdr[:, c0:c1],
                                         start=True, stop=True,
                                         tile_position=(p0, 0))
                        nc.tensor.matmul(Ah0[:, c0:c1], aL, wdr[:, c0:c1],
                                         start=True, stop=True,
                                         tile_position=(p0, 0))
                        nc.tensor.matmul(Eh1[:, c0:c1], eL, wdr[:, H + c0:H + c1],
                                         start=True, stop=True,
                                         tile_position=(p0, 0))
                        nc.tensor.matmul(Ah1[:, c0:c1], aL, wdr[:, H + c0:H + c1],
                                         start=True, stop=True,
                                         tile_position=(p0, 0))

                    S0 = sul.tile([128, H], F16, tag="S0")
                    nc.scalar.activation(S0[:, :], Eh0[:, :], ACT.Identity, bias=1.0)
                    A0 = sul.tile([128, H], F16, tag="A0")
                    nc.scalar.activation(A0[:, :], Ah0[:, :], ACT.Copy, bias=0.0)
                    u0 = sul.tile([128, H], F16, tag="u0")
                    nc.vector.tensor_tensor(u0[:, :], m_sb[:, 0:H], S0[:, :], ALU.mult)
                    nc.gpsimd.tensor_tensor(m_sb[:, 0:H], u0[:, :], A0[:, :], ALU.add)

                    u1 = sul.tile([128, H], F16, tag="u1")
                    nc.vector.scalar_tensor_tensor(u1[:, :], Eh1[:, :], 1.0,
                                                   m_sb[:, H:COLS],
                                                   ALU.add, ALU.mult)
                    nc.vector.tensor_tensor(m_sb[:, H:COLS], u1[:, :], Ah1[:, :],
                                            ALU.add)

            # ---------- readout + head ----------
            with tc.tile_pool(name="psr", bufs=1, space="PSUM") as psr, \
                 tc.tile_pool(name="sbr", bufs=1) as sbr:
                # target question embedding, transposed
                qv32 = sbr.tile([BL, KD], F32, tag="qv32")
                nc.gpsimd.indirect_dma_start(
                    out=qv32[:, :], out_offset=None,
                    in_=q_emb[:, :],
                    in_offset=bass.IndirectOffsetOnAxis(ap=idx_t_sb[:, 0:1], axis=0),
                )
                qv16 = sbr.tile([BL, KD], F16, tag="qv16")
                nc.vector.tensor_copy(qv16[:, :], qv32[:, :])
                qvT_ps = psr.tile([KD, BL], F16, tag="psmall")
                nc.tensor.transpose(qvT_ps[:, :], qv16[:, :], w_id[:BL, :BL])
                qvT = sbr.tile([KD, BL], F16, tag="qvT")
                nc.vector.tensor_copy(qvT[:, :], qvT_ps[:, :])

                # wt = softmax(qv @ key_W.T)
                zt = psr.tile([BL, C], F32, tag="psmall")
                nc.tensor.matmul(zt[:, :], qvT[:, :], w_kWt[:, :], start=True, stop=True)
                mxt = sbr.tile([BL, 1], F32, tag="mxt")
                nc.vector.tensor_reduce(mxt[:, :], zt[:, :], AX.X, ALU.max)
                nmxt = sbr.tile([BL, 1], F32, tag="nmxt")
                nc.vector.tensor_scalar_mul(nmxt[:, :], mxt[:, :], -1.0)
                wext = sbr.tile([BL, C], F32, tag="wext")
                nc.scalar.activation(wext[:, :], zt[:, :], ACT.Exp, bias=nmxt[:, :])
                smt = sbr.tile([BL, 1], F32, tag="smt")
                nc.vector.tensor_reduce(smt[:, :], wext[:, :], AX.X, ALU.add)
                rct = sbr.tile([BL, 1], F32, tag="rct")
                nc.vector.reciprocal(rct[:, :], smt[:, :])
                wt16 = sbr.tile([BL, C], F16, tag="wt16")
                nc.vector.tensor_scalar_mul(wt16[:, :], wext[:, :], rct[:, :])
                # flatten to (1, 2048): [right blocks | wrong blocks], both = wt
                wtf = sbr.tile([1, COLS], F16, tag="wtf")
                nc.gpsimd.dma_start(out=wtf[0:1, 0:BL * C], in_=wt16[:, :])
                nc.gpsimd.dma_start(out=wtf[0:1, BL * C:COLS], in_=wt16[:, :])
                # broadcast over partitions via K=1 matmul
                wb_ps = psr.tile([128, COLS], F32, tag="wb_ps")
                for k in range(4):
                    nc.tensor.matmul(wb_ps[:, 512 * k:512 * (k + 1)], w_ones[:, :],
                                     wtf[:, 512 * k:512 * (k + 1)],
                                     start=True, stop=True)
                wb = sbr.tile([128, COLS], F16, tag="wb")
                nc.scalar.activation(wb[:, :], wb_ps[:, :], ACT.Copy, bias=0.0)
                u2 = sbr.tile([128, COLS], F16, tag="u2")
                nc.vector.tensor_tensor(u2[:, :], m_sb[:, :], wb[:, :], ALU.mult)
                rr = sbr.tile([VD, NR], F32, tag="rr")
                nc.vector.tensor_reduce(
                    rr[:, :], u2[:].rearrange("p (r c) -> p r c", c=C), AX.X, ALU.add)
                rr16 = sbr.tile([VD, NR], F16, tag="rr16")
                nc.vector.tensor_copy(rr16[:, :], rr[:, :])

                # r_sum / w_sum: (SD, BL)
                rs_ps = psr.tile([SD, BL], F32, tag="psmall")
                nc.tensor.matmul(rs_ps[:, :], w_rs0[:, :], rr16[:, 0:BL],
                                 start=True, stop=False)
                nc.tensor.matmul(rs_ps[:, :], w_rs1[:, :], qvT[:, :],
                                 start=False, stop=True)
                rsum = sbr.tile([SD, BL], F16, tag="rsum")
                nc.scalar.activation(rsum[:, :], rs_ps[:, :], ACT.Tanh,
                                     bias=w_rsb[:, :])
                ws_ps = psr.tile([SD, BL], F32, tag="psmall")
                nc.tensor.matmul(ws_ps[:, :], w_ws0[:, :], rr16[:, BL:NR],
                                 start=True, stop=False)
                nc.tensor.matmul(ws_ps[:, :], w_ws1[:, :], qvT[:, :],
                                 start=False, stop=True)
                wsum = sbr.tile([SD, BL], F16, tag="wsum")
                nc.scalar.activation(wsum[:, :], ws_ps[:, :], ACT.Tanh,
                                     bias=w_wsb[:, :])

                # success/failure/difficulty levels: (1, BL)
                lv_ps = psr.tile([1, BL], F32, tag="psmall")
                succ = sbr.tile([1, BL], F32, tag="succ")
                nc.tensor.matmul(lv_ps[:, :], w_succ[:, :], rsum[:, :],
                                 start=True, stop=True)
                nc.scalar.activation(succ[:, :], lv_ps[:, :], ACT.Tanh,
                                     bias=w_sb_b[:, 0:1])
                lv_ps2 = psr.tile([1, BL], F32, tag="psmall")
                fail = sbr.tile([1, BL], F32, tag="fail")
                nc.tensor.matmul(lv_ps2[:, :], w_fail[:, :], wsum[:, :],
                                 start=True, stop=True)
                nc.scalar.activation(fail[:, :], lv_ps2[:, :], ACT.Tanh,
                                     bias=w_sb_b[:, 1:2])
                lv_ps3 = psr.tile([1, BL], F32, tag="psmall")
                diff = sbr.tile([1, BL], F32, tag="diff")
                nc.tensor.matmul(lv_ps3[:, :], w_diff[:, :], qvT[:, :],
                                 start=True, stop=True)
                nc.scalar.activation(diff[:, :], lv_ps3[:, :], ACT.Tanh,
                                     bias=w_sb_b[:, 2:3])

                # global success/failure counts (use FULL inputs, same all cores)
                sigs = sbr.tile([1, 2], F32, tag="sigs")
                for ci, full in enumerate([right_full, wrong_full]):
                    fin = sbr.tile([B, S], I32, tag="fin")
                    nc.sync.dma_start(out=fin[:, :], in_=full[:, :])
                    ff = sbr.tile([B, S], F32, tag="ff")
                    nc.vector.tensor_copy(ff[:, :], fin[:, :])
                    fc = sbr.tile([B, S], F32, tag="fc")
                    nc.vector.tensor_scalar(fc[:, :], ff[:, :], 1.0, None, ALU.min)
                    cs = sbr.tile([B, 1], F32, tag="cs")
                    nc.vector.tensor_reduce(cs[:, :], fc[:, :], AX.X, ALU.add)
                    cnt_ps = psr.tile([1, 1], F32, tag="psmall")
                    nc.tensor.matmul(cnt_ps[:, :], cs[:, :], w_ones_c32[:, :],
                                     start=True, stop=True)
                    nc.scalar.activation(sigs[:, ci:ci + 1], cnt_ps[:, :], ACT.Sigmoid)

                # out = succ*sig(sc) + fail*sig(fc) - 2*diff
                t1 = sbr.tile([1, BL], F32, tag="t1")
                nc.vector.tensor_scalar_mul(t1[:, :], succ[:, :], sigs[:, 0:1])
                t2 = sbr.tile([1, BL], F32, tag="t2")
                nc.vector.tensor_scalar_mul(t2[:, :], fail[:, :], sigs[:, 1:2])
                t3 = sbr.tile([1, BL], F32, tag="t3")
                nc.vector.tensor_scalar_mul(t3[:, :], diff[:, :], -2.0)
                o1 = sbr.tile([1, BL], F32, tag="o1")
                nc.vector.tensor_tensor(o1[:, :], t1[:, :], t2[:, :], ALU.add)
                o2 = sbr.tile([1, BL], F32, tag="o2")
                nc.vector.tensor_tensor(o2[:, :], o1[:, :], t3[:, :], ALU.add)
                nc.sync.dma_start(out=out_d[:, :], in_=o2[:, :])

                if DEBUG:
                    nc.sync.dma_start(out=dbg_m[:, :], in_=m_sb[:, :])
                    nc.sync.dma_start(out=dbg_e[:, :], in_=e_all[:, 0:S])
                    nc.sync.dma_start(out=dbg_na[:, :], in_=na_all[:, 0:S])
                    nc.sync.dma_start(out=dbg_rr[:, :], in_=rr[:, :])
                    nc.sync.dma_start(
                        out=dbg_wd[:, :],
                        in_=bass.AP(wd_dram, 0, [[2048, 32], [1, 2048]]))

    nc.compile()
    return nc


_PROGRAM = None


def _get_program():
    global _PROGRAM
    if _PROGRAM is None:
        _PROGRAM = _build_program()
    return _PROGRAM


def _host_inputs(inputs):
    """Build the per-core in_maps from the full problem inputs."""
    f16 = np.float16
    f32 = np.float32
    ri = np.asarray(inputs["right_input"]).astype(np.int64)
    wi = np.asarray(inputs["wrong_input"]).astype(np.int64)
    tg = np.asarray(inputs["target_id"]).astype(np.int64)
    q_emb = np.asarray(inputs["q_emb"], dtype=f32)
    i_emb = np.asarray(inputs["i_emb"], dtype=f32)

    def W(name):
        return np.asarray(inputs[name], dtype=f32)

    common = {
        "i_emb": i_emb,
        "q_emb": q_emb,
        "erase_Wt": np.ascontiguousarray(W("erase_W").T).astype(f16),
        "add_Wt": np.ascontiguousarray(W("add_W").T).astype(f16),
        "key_Wt": np.ascontiguousarray(W("key_W").T).astype(f16),
        "erase_b_row": W("erase_b").reshape(1, -1).astype(f16),
        "add_b_row": W("add_b").reshape(1, -1).astype(f16),
        "rsum_Wt0": np.ascontiguousarray(W("rsum_W")[:, :VD].T).astype(f16),
        "rsum_Wt1": np.ascontiguousarray(W("rsum_W")[:, VD:].T).astype(f16),
        "wsum_Wt0": np.ascontiguousarray(W("wsum_W")[:, :VD].T).astype(f16),
        "wsum_Wt1": np.ascontiguousarray(W("wsum_W")[:, VD:].T).astype(f16),
        "rsum_b_col": W("rsum_b").reshape(-1, 1).astype(f32),
        "wsum_b_col": W("wsum_b").reshape(-1, 1).astype(f32),
        "succ_Wt": np.ascontiguousarray(W("succ_W").T).astype(f16),
        "fail_Wt": np.ascontiguousarray(W("fail_W").T).astype(f16),
        "diff_Wt": np.ascontiguousarray(W("diff_W").T).astype(f16),
        "succ_b": W("succ_b").reshape(1, 1).astype(f32),
        "fail_b": W("fail_b").reshape(1, 1).astype(f32),
        "diff_b": W("diff_b").reshape(1, 1).astype(f32),
        "rmem0": W("right_mem_init").astype(f16),
        "wmem0": W("wrong_mem_init").astype(f16),
        "ones_row": np.ones((1, 128), dtype=f16),
        "ones_col32": np.ones((128, 1), dtype=f32),
        "id128": np.eye(128, dtype=f16),
        "right_full": ri.astype(np.int32),
        "wrong_full": wi.astype(np.int32),
    }

    in_maps = []
    for core in range(NCORE):
        rows = slice(core * BL, (core + 1) * BL)
        # inter ids per (t, r): r<BL -> right, else wrong
        inter = np.empty((S, NR), dtype=np.int64)
        inter[:, :BL] = ri[rows].T
        inter[:, BL:] = wi[rows].T
        qid = inter - Q * (inter > Q)
        flat_i = inter.reshape(-1)
        flat_q = qid.reshape(-1)
        idx_i = flat_i.reshape(NG, 128).T.astype(np.int32)
        idx_q = flat_q.reshape(NG, 128).T.astype(np.int32)
        idx_t = tg[rows].reshape(BL, 1).astype(np.int32)
        in_maps.append({**common, "idx_i": np.ascontiguousarray(idx_i),
                        "idx_q": np.ascontiguousarray(idx_q),
                        "idx_t": idx_t})
    return in_maps


def run_spmd(inputs, trace=False):
    nc = _get_program()
    in_maps = _host_inputs(inputs)
    res = run_bass_kernel_spmd(nc, in_maps, core_ids=list(range(NCORE)),
                               trace=trace)
    out = np.concatenate([res.results[i]["out"] for i in range(NCORE)], axis=0)
    return out.astype(np.float32), res


def kernel(**inputs):
    out, _ = run_spmd(inputs, trace=False)
    return out


# revision 11
# speedup vs baseline: 1.3208x; 1.3208x over previous
"""DiKT (DKVMN-style knowledge tracing) Trainium2 kernel.

Self-contained: builds a Bass/Tile program, shards batch over 8 NeuronCores
(pure data parallel, 16 batch rows per core), runs via run_bass_kernel_spmd.

Algorithm per core (B_loc=16, V=128, C=64, S=128 steps):
  Both value memories (right/wrong) live as ONE SBUF tensor m[v=128, col=2048]
  with col = r*64 + c, r = mem*16 + b.  Per step:
     m' = m * (1 - e x w) + a x w

Two consecutive steps are FUSED into one update (64 pairs):
     Shat = S1*S2 = 1 - e1 x w1 - e2 x w2 + (e1e2) x (w1w2)
     Ahat = S2*A1 + A2 = a1 x w1 - (a1e2) x (w1w2) + a2 x w2
Both are rank-3 sums of outer products, built by ONE K=96 TensorE matmul
against a 3-group block-diagonal rhs (per group g, row r: the (r,c) diagonal
block carries [-w1 | -w2 | +w1w2]).  lhsT groups: [e1 | e2 | e1e2] for Shat,
[-a1 | -a2 | -a1e2] for Ahat (signs make every product come out right).

Consumption per pair (m' = m*(1+Shat') + Ahat, Shat' = Shat-1 from PE):
  right memory (cols 0:1024):  ACT evacuates 1+Shat' and Ahat to fp16 SBUF,
     DVE multiplies, GpSimd adds.
  wrong memory (cols 1024:2048): DVE scalar_tensor_tensor (+1, mult) straight
     from PSUM, DVE add straight from PSUM.

All per-step e, a, w are precomputed up front from embedding gathers
(indirect DMA) + small matmuls; the fused block-diagonal rhs for all pairs
is staged in DRAM and streamed during the loop.
"""

import numpy as np

import concourse.mybir as mybir
from concourse import bass, bacc, tile
from concourse.bass_utils import run_bass_kernel_spmd

F16 = mybir.dt.float16
F32 = mybir.dt.float32
I32 = mybir.dt.int32
ALU = mybir.AluOpType
ACT = mybir.ActivationFunctionType
AX = mybir.AxisListType

# model dims
KD = 128      # KEY_DIM
VD = 128      # VALUE_DIM
SD = 128      # SUMMARY_DIM
Q = 10000     # QUESTION_NUM
C = 64        # CONCEPT_NUM
B = 128       # full batch
S = 128       # seq len
NCORE = 8
BL = B // NCORE          # 16 batch rows per core
NR = 2 * BL              # 32 rows per step (right+wrong)
COLS = NR * C            # 2048 memory columns per core
NP = S // 2              # 64 step pairs
KF = 3 * NR              # 96 contraction rows per fused matmul
PB = KF * COLS           # 196608 elements per pair block in wd_dram
NGQ = (S * NR) // 128    # 32 q-side gather chunks of 128 rows

DEBUG = False


def _build_program():
    nc = bacc.Bacc(trn_type="TRN2", target_bir_lowering=False, num_devices=NCORE,
                   num_swdge_queues=4)

    # ---- DRAM inputs ----
    i_emb = nc.dram_tensor("i_emb", [2 * Q + 1, VD], F32, kind="ExternalInput")
    q_emb = nc.dram_tensor("q_emb", [Q + 1, KD], F32, kind="ExternalInput")
    idx_i = nc.dram_tensor("idx_i", [KF, NP], I32, kind="ExternalInput")
    idx_q = nc.dram_tensor("idx_q", [128, NGQ], I32, kind="ExternalInput")
    idx_t = nc.dram_tensor("idx_t", [BL, 1], I32, kind="ExternalInput")

    erase_Wt = nc.dram_tensor("erase_Wt", [VD, VD], F16, kind="ExternalInput")
    add_Wt = nc.dram_tensor("add_Wt", [VD, VD], F16, kind="ExternalInput")
    key_Wt = nc.dram_tensor("key_Wt", [KD, C], F16, kind="ExternalInput")
    erase_b_row = nc.dram_tensor("erase_b_row", [1, VD], F16, kind="ExternalInput")
    add_b_row = nc.dram_tensor("add_b_row", [1, VD], F16, kind="ExternalInput")
    rsum_Wt0 = nc.dram_tensor("rsum_Wt0", [VD, SD], F16, kind="ExternalInput")
    rsum_Wt1 = nc.dram_tensor("rsum_Wt1", [KD, SD], F16, kind="ExternalInput")
    wsum_Wt0 = nc.dram_tensor("wsum_Wt0", [VD, SD], F16, kind="ExternalInput")
    wsum_Wt1 = nc.dram_tensor("wsum_Wt1", [KD, SD], F16, kind="ExternalInput")
    rsum_b_col = nc.dram_tensor("rsum_b_col", [SD, 1], F32, kind="ExternalInput")
    wsum_b_col = nc.dram_tensor("wsum_b_col", [SD, 1], F32, kind="ExternalInput")
    succ_Wt = nc.dram_tensor("succ_Wt", [SD, 1], F16, kind="ExternalInput")
    fail_Wt = nc.dram_tensor("fail_Wt", [SD, 1], F16, kind="ExternalInput")
    diff_Wt = nc.dram_tensor("diff_Wt", [KD, 1], F16, kind="ExternalInput")
    succ_b = nc.dram_tensor("succ_b", [1, 1], F32, kind="ExternalInput")
    fail_b = nc.dram_tensor("fail_b", [1, 1], F32, kind="ExternalInput")
    diff_b = nc.dram_tensor("diff_b", [1, 1], F32, kind="ExternalInput")
    rmem0 = nc.dram_tensor("rmem0", [VD, C], F16, kind="ExternalInput")
    wmem0 = nc.dram_tensor("wmem0", [VD, C], F16, kind="ExternalInput")
    ones_row = nc.dram_tensor("ones_row", [1, 128], F16, kind="ExternalInput")
    ones_col32 = nc.dram_tensor("ones_col32", [128, 1], F32, kind="ExternalInput")
    id128 = nc.dram_tensor("id128", [128, 128], F16, kind="ExternalInput")
    right_full = nc.dram_tensor("right_full", [B, S], I32, kind="ExternalInput")
    wrong_full = nc.dram_tensor("wrong_full", [B, S], I32, kind="ExternalInput")

    out_d = nc.dram_tensor("out", [BL, 1], F32, kind="ExternalOutput")
    if DEBUG:
        dbg_m = nc.dram_tensor("dbg_m", [VD, COLS], F16, kind="ExternalOutput")
        dbg_rr = nc.dram_tensor("dbg_rr", [VD, NR], F32, kind="ExternalOutput")

    # fused block-diagonal rhs for every pair, flat fp16:
    # pair p, group g (0..2), row r (0..31): diag block at
    #   p*PB + g*65536 + r*2112, 64 wide
    wd_dram = nc.dram_tensor("wd_dram", [NP * PB], F16)

    # ---- persistent SBUF ----
    sb = lambda name, shape, dt: nc.alloc_sbuf_tensor(name, shape, dt)
    m_sb = sb("m_sb", [VD, COLS], F16)
    vecT = sb("vecT", [128, NP * KF], F16)   # i_emb rows (pair layout), transposed
    qT = sb("qT", [128, NGQ * 128], F16)
    eP = sb("eP", [KF, NP * 128], F16)       # [e1 | e2 | e1e2] per pair
    naP = sb("naP", [KF, NP * 128], F16)     # [-a1 | -a2 | -a1e2] per pair
    scr = sb("scr", [KF, 128], F16)          # e2 staging at group-2 partitions
    wstage = sb("wstage", [128, C], F16)     # w2 staging at w1's partitions
    wprod = sb("wprod", [128, C], F16)       # w1*w2
    w_eWt = sb("w_eWt", [VD, VD], F16)
    w_aWt = sb("w_aWt", [VD, VD], F16)
    w_kWt = sb("w_kWt", [KD, C], F16)
    w_eb = sb("w_eb", [1, VD], F16)
    w_ab = sb("w_ab", [1, VD], F16)
    w_ones = sb("w_ones", [1, 128], F16)
    w_ones_c32 = sb("w_ones_c32", [128, 1], F32)
    w_id = sb("w_id", [128, 128], F16)
    idx_i_sb = sb("idx_i_sb", [KF, NP], I32)
    idx_q_sb = sb("idx_q_sb", [128, NGQ], I32)
    idx_t_sb = sb("idx_t_sb", [BL, 1], I32)
    w_rs0 = sb("w_rs0", [VD, SD], F16)
    w_rs1 = sb("w_rs1", [KD, SD], F16)
    w_ws0 = sb("w_ws0", [VD, SD], F16)
    w_ws1 = sb("w_ws1", [KD, SD], F16)
    w_rsb = sb("w_rsb", [SD, 1], F32)
    w_wsb = sb("w_wsb", [SD, 1], F32)
    w_succ = sb("w_succ", [SD, 1], F16)
    w_fail = sb("w_fail", [SD, 1], F16)
    w_diff = sb("w_diff", [KD, 1], F16)
    w_sb_b = sb("w_sb_b", [1, 3], F32)  # succ_b, fail_b, diff_b columns 0..2
    zeros2k = sb("zeros2k", [128, COLS], F16)

    with tile.TileContext(nc) as tc:
        with tc.tile_pool(name="sbp", bufs=3) as sbp:
            # ---------- load constants ----------
            for dst, src in [
                (w_eWt, erase_Wt), (w_aWt, add_Wt), (w_kWt, key_Wt),
                (w_eb, erase_b_row), (w_ab, add_b_row), (w_ones, ones_row),
                (w_ones_c32, ones_col32), (w_id, id128),
                (idx_i_sb, idx_i), (idx_q_sb, idx_q), (idx_t_sb, idx_t),
                (w_rs0, rsum_Wt0), (w_rs1, rsum_Wt1),
                (w_ws0, wsum_Wt0), (w_ws1, wsum_Wt1),
                (w_rsb, rsum_b_col), (w_wsb, wsum_b_col),
                (w_succ, succ_Wt), (w_fail, fail_Wt), (w_diff, diff_Wt),
            ]:
                nc.sync.dma_start(out=dst[:, :], in_=src[:, :])
            nc.sync.dma_start(out=w_sb_b[:, 0:1], in_=succ_b[:, :])
            nc.sync.dma_start(out=w_sb_b[:, 1:2], in_=fail_b[:, :])
            nc.sync.dma_start(out=w_sb_b[:, 2:3], in_=diff_b[:, :])

            # zero-fill wd_dram (24 MiB fp16) from a zeroed sbuf tile
            nc.gpsimd.memset(zeros2k[:, :], 0.0)
            for g in range(NP * PB // (128 * 2048)):
                nc.scalar.dma_start(
                    out=bass.AP(wd_dram, g * 128 * 2048, [[2048, 128], [1, 2048]]),
                    in_=zeros2k[:, :],
                )

            # init m: broadcast mem inits over the 16 batch blocks
            rmem_t = sbp.tile([VD, C], F16, tag="memi")
            nc.sync.dma_start(out=rmem_t[:, :], in_=rmem0[:, :])
            wmem_t = sbp.tile([VD, C], F16, tag="memi2")
            nc.sync.dma_start(out=wmem_t[:, :], in_=wmem0[:, :])
            for r in range(NR):
                srct = rmem_t if r < BL else wmem_t
                nc.vector.tensor_copy(m_sb[:, r * C:(r + 1) * C], srct[:, :])

            # ---------- gathers + transposes ----------
            with tc.tile_pool(name="pst", bufs=2, space="PSUM") as psp:
                for p in range(NP):
                    lo = p * KF
                    gi32 = sbp.tile([KF, VD], F32, tag="gi32")
                    nc.gpsimd.indirect_dma_start(
                        out=gi32[:, :], out_offset=None,
                        in_=i_emb[:, :],
                        in_offset=bass.IndirectOffsetOnAxis(
                            ap=idx_i_sb[:, p:p + 1], axis=0),
                    )
                    gi16 = sbp.tile([KF, VD], F16, tag="gi16")
                    nc.vector.tensor_copy(gi16[:, :], gi32[:, :])
                    tps = psp.tile([128, KF], F16, tag="tp")
                    nc.tensor.transpose(tps[:, :], gi16[:, :], w_id[:KF, :KF])
                    nc.vector.tensor_copy(vecT[:, lo:lo + KF], tps[:, :])
                for g in range(NGQ):
                    lo = g * 128
                    gq32 = sbp.tile([128, KD], F32, tag="gq32")
                    nc.gpsimd.indirect_dma_start(
                        out=gq32[:, :], out_offset=None,
                        in_=q_emb[:, :],
                        in_offset=bass.IndirectOffsetOnAxis(
                            ap=idx_q_sb[:, g:g + 1], axis=0),
                    )
                    gq16 = sbp.tile([128, KD], F16, tag="gq16")
                    nc.vector.tensor_copy(gq16[:, :], gq32[:, :])
                    tps2 = psp.tile([128, 128], F16, tag="tp2")
                    nc.tensor.transpose(tps2[:, :], gq16[:, :], w_id[:, :])
                    nc.vector.tensor_copy(qT[:, lo:lo + 128], tps2[:, :])

            # ---------- e/a precompute (pair layout) ----------
            # grouped by ACT table set: all Sigmoid, then all Tanh, then Exp
            with tc.tile_pool(name="psz", bufs=2, space="PSUM") as psp:
                for pg in range(NP // 4):
                    eps = psp.tile([KF, 512], F32, tag="eps")
                    for k in range(4):
                        p = 4 * pg + k
                        nc.tensor.matmul(eps[:, 128 * k:128 * (k + 1)],
                                         vecT[:, p * KF:(p + 1) * KF], w_eWt[:, :],
                                         start=True, stop=False)
                        nc.tensor.matmul(eps[:, 128 * k:128 * (k + 1)],
                                         w_ones[:, :KF], w_eb[:, :],
                                         start=False, stop=True)
                    nc.scalar.activation(eP[:, 512 * pg:512 * (pg + 1)],
                                         eps[:, :], ACT.Sigmoid)
                for pg in range(NP // 4):
                    aps = psp.tile([KF, 512], F32, tag="aps")
                    for k in range(4):
                        p = 4 * pg + k
                        nc.tensor.matmul(aps[:, 128 * k:128 * (k + 1)],
                                         vecT[:, p * KF:(p + 1) * KF], w_aWt[:, :],
                                         start=True, stop=False)
                        nc.tensor.matmul(aps[:, 128 * k:128 * (k + 1)],
                                         w_ones[:, :KF], w_ab[:, :],
                                         start=False, stop=True)
                    nc.scalar.activation(naP[:, 512 * pg:512 * (pg + 1)],
                                         aps[:, :], ACT.Tanh, scale=-1.0)
                # group-2 products: copy e2 to group-2 partitions, multiply
                for p in range(NP):
                    lo = p * 128
                    nc.sync.dma_start(out=scr[64:96, :],
                                      in_=eP[32:64, lo:lo + 128])
                    nc.vector.tensor_tensor(naP[64:96, lo:lo + 128],
                                            naP[64:96, lo:lo + 128],
                                            scr[64:96, :], ALU.mult)
                    nc.vector.tensor_tensor(eP[64:96, lo:lo + 128],
                                            eP[64:96, lo:lo + 128],
                                            scr[64:96, :], ALU.mult)

                # w = softmax(qv @ key_W.T) (negated); pair products; scatter
                for g in range(NGQ):
                    lo = g * 128
                    zps = psp.tile([128, C], F32, tag="zps")
                    nc.tensor.matmul(zps[:, :], qT[:, lo:lo + 128], w_kWt[:, :],
                                     start=True, stop=True)
                    wex = sbp.tile([128, C], F32, tag="wex")
                    nc.scalar.activation(wex[:, :], zps[:, :], ACT.Exp)
                    sm = sbp.tile([128, 1], F32, tag="sm")
                    nc.vector.tensor_reduce(sm[:, :], wex[:, :], AX.X, ALU.add)
                    rc = sbp.tile([128, 1], F32, tag="rc")
                    nc.vector.reciprocal(rc[:, :], sm[:, :])
                    wng = sbp.tile([128, C], F16, tag="wng")
                    nc.vector.tensor_scalar(wng[:, :], wex[:, :], rc[:, :], -1.0,
                                            ALU.mult, ALU.mult)
                    # two pairs per 128-row chunk: rows [0:32]=w1,[32:64]=w2 etc
                    for half in range(2):
                        p = 2 * g + half
                        r0 = 64 * half
                        nc.sync.dma_start(out=wstage[r0:r0 + 32, :],
                                          in_=wng[r0 + 32:r0 + 64, :])
                        nc.vector.tensor_tensor(wprod[r0:r0 + 32, :],
                                                wng[r0:r0 + 32, :],
                                                wstage[r0:r0 + 32, :], ALU.mult)
                        base = p * PB
                        for gi, src_t, srow in (
                            (0, wng, r0), (1, wng, r0 + 32), (2, wprod, r0)):
                            nc.sync.dma_start(
                                out=bass.AP(wd_dram, base + gi * 65536,
                                            [[COLS + C, NR], [1, C]]),
                                in_=src_t[srow:srow + 32, :],
                            )

            # ---------- the fused pair recurrence ----------
            H = COLS // 2
            with tc.tile_pool(name="psl", bufs=1, space="PSUM") as psl, \
                 tc.tile_pool(name="wdp", bufs=2) as wdp, \
                 tc.tile_pool(name="sul", bufs=2) as sul:
                for p in range(NP):
                    wd4 = wdp.tile([KF, COLS], F16, tag="wd4")
                    nc.sync.dma_start(
                        out=wd4[:, :],
                        in_=bass.AP(wd_dram, p * PB, [[2048, KF], [1, 2048]]),
                    )
                    eL = eP[:, p * 128:(p + 1) * 128]
                    aL = naP[:, p * 128:(p + 1) * 128]
                    Eh0 = psl.tile([128, H], F32, tag="Eh0")
                    Ah0 = psl.tile([128, H], F32, tag="Ah0")
                    Eh1 = psl.tile([128, H], F32, tag="Eh1")
                    Ah1 = psl.tile([128, H], F32, tag="Ah1")
                    for hk in range(2):
                        c0, c1 = 512 * hk, 512 * (hk + 1)
                        nc.tensor.matmul(Eh1[:, c0:c1], eL, wd4[:, H + c0:H + c1],
                                         start=True, stop=True)
                        nc.tensor.matmul(Ah1[:, c0:c1], aL, wd4[:, H + c0:H + c1],
                                         start=True, stop=True)
                    for hk in range(2):
                        c0, c1 = 512 * hk, 512 * (hk + 1)
                        nc.tensor.matmul(Eh0[:, c0:c1], eL, wd4[:, c0:c1],
                                         start=True, stop=True)
                        nc.tensor.matmul(Ah0[:, c0:c1], aL, wd4[:, c0:c1],
                                         start=True, stop=True)
                    # wrong memory: PSUM-direct on DVE
                    u1 = sul.tile([128, H], F16, tag="u1")
                    nc.vector.scalar_tensor_tensor(u1[:, :], Eh1[:, :], 1.0,
                                                   m_sb[:, H:COLS],
                                                   ALU.add, ALU.mult)
                    nc.vector.tensor_tensor(m_sb[:, H:COLS], u1[:, :], Ah1[:, :],
                                            ALU.add)
                    # right memory: ACT-evacuated, DVE mul, POOL add
                    S0 = sul.tile([128, H], F16, tag="S0")
                    nc.scalar.activation(S0[:, :], Eh0[:, :], ACT.Identity,
                                         bias=1.0)
                    A0 = sul.tile([128, H], F16, tag="A0")
                    nc.scalar.activation(A0[:, :], Ah0[:, :], ACT.Copy, bias=0.0)
                    u0 = sul.tile([128, H], F16, tag="u0")
                    nc.vector.tensor_tensor(u0[:, :], m_sb[:, 0:H], S0[:, :],
                                            ALU.mult)
                    nc.gpsimd.tensor_tensor(m_sb[:, 0:H], u0[:, :], A0[:, :],
                                            ALU.add)

            # ---------- readout + head ----------
            with tc.tile_pool(name="psr", bufs=1, space="PSUM") as psr, \
                 tc.tile_pool(name="sbr", bufs=1) as sbr:
                # target question embedding, transposed
                qv32 = sbr.tile([BL, KD], F32, tag="qv32")
                nc.gpsimd.indirect_dma_start(
                    out=qv32[:, :], out_offset=None,
                    in_=q_emb[:, :],
                    in_offset=bass.IndirectOffsetOnAxis(ap=idx_t_sb[:, 0:1], axis=0),
                )
                qv16 = sbr.tile([BL, KD], F16, tag="qv16")
                nc.vector.tensor_copy(qv16[:, :], qv32[:, :])
                qvT_ps = psr.tile([KD, BL], F16, tag="psmall")
                nc.tensor.transpose(qvT_ps[:, :], qv16[:, :], w_id[:BL, :BL])
                qvT = sbr.tile([KD, BL], F16, tag="qvT")
                nc.vector.tensor_copy(qvT[:, :], qvT_ps[:, :])

                # wt = softmax(qv @ key_W.T)
                zt = psr.tile([BL, C], F32, tag="psmall")
                nc.tensor.matmul(zt[:, :], qvT[:, :], w_kWt[:, :], start=True,
                                 stop=True)
                wext = sbr.tile([BL, C], F32, tag="wext")
                nc.scalar.activation(wext[:, :], zt[:, :], ACT.Exp)
                smt = sbr.tile([BL, 1], F32, tag="smt")
                nc.vector.tensor_reduce(smt[:, :], wext[:, :], AX.X, ALU.add)
                rct = sbr.tile([BL, 1], F32, tag="rct")
                nc.vector.reciprocal(rct[:, :], smt[:, :])
                wt16 = sbr.tile([BL, C], F16, tag="wt16")
                nc.vector.tensor_scalar_mul(wt16[:, :], wext[:, :], rct[:, :])
                # flatten to (1, 2048): [right blocks | wrong blocks], both = wt
                wtf = sbr.tile([1, COLS], F16, tag="wtf")
                nc.gpsimd.dma_start(out=wtf[0:1, 0:BL * C], in_=wt16[:, :])
                nc.gpsimd.dma_start(out=wtf[0:1, BL * C:COLS], in_=wt16[:, :])
                # broadcast over partitions via K=1 matmul
                wb_ps = psr.tile([128, COLS], F32, tag="wb_ps")
                for k in range(4):
                    nc.tensor.matmul(wb_ps[:, 512 * k:512 * (k + 1)], w_ones[:, :],
                                     wtf[:, 512 * k:512 * (k + 1)],
                                     start=True, stop=True)
                wb = sbr.tile([128, COLS], F16, tag="wb")
                nc.scalar.activation(wb[:, :], wb_ps[:, :], ACT.Copy, bias=0.0)
                u2 = sbr.tile([128, COLS], F16, tag="u2")
                nc.vector.tensor_tensor(u2[:, :], m_sb[:, :], wb[:, :], ALU.mult)
                rr = sbr.tile([VD, NR], F32, tag="rr")
                nc.vector.tensor_reduce(
                    rr[:, :], u2[:].rearrange("p (r c) -> p r c", c=C), AX.X,
                    ALU.add)
                rr16 = sbr.tile([VD, NR], F16, tag="rr16")
                nc.vector.tensor_copy(rr16[:, :], rr[:, :])

                # r_sum / w_sum: (SD, BL)
                rs_ps = psr.tile([SD, BL], F32, tag="psmall")
                nc.tensor.matmul(rs_ps[:, :], w_rs0[:, :], rr16[:, 0:BL],
                                 start=True, stop=False)
                nc.tensor.matmul(rs_ps[:, :], w_rs1[:, :], qvT[:, :],
                                 start=False, stop=True)
                rsum = sbr.tile([SD, BL], F16, tag="rsum")
                nc.scalar.activation(rsum[:, :], rs_ps[:, :], ACT.Tanh,
                                     bias=w_rsb[:, :])
                ws_ps = psr.tile([SD, BL], F32, tag="psmall")
                nc.tensor.matmul(ws_ps[:, :], w_ws0[:, :], rr16[:, BL:NR],
                                 start=True, stop=False)
                nc.tensor.matmul(ws_ps[:, :], w_ws1[:, :], qvT[:, :],
                                 start=False, stop=True)
                wsum = sbr.tile([SD, BL], F16, tag="wsum")
                nc.scalar.activation(wsum[:, :], ws_ps[:, :], ACT.Tanh,
                                     bias=w_wsb[:, :])

                # success/failure/difficulty levels: (1, BL)
                lv_ps = psr.tile([1, BL], F32, tag="psmall")
                succ = sbr.tile([1, BL], F32, tag="succ")
                nc.tensor.matmul(lv_ps[:, :], w_succ[:, :], rsum[:, :],
                                 start=True, stop=True)
                nc.scalar.activation(succ[:, :], lv_ps[:, :], ACT.Tanh,
                                     bias=w_sb_b[:, 0:1])
                lv_ps2 = psr.tile([1, BL], F32, tag="psmall")
                fail = sbr.tile([1, BL], F32, tag="fail")
                nc.tensor.matmul(lv_ps2[:, :], w_fail[:, :], wsum[:, :],
                                 start=True, stop=True)
                nc.scalar.activation(fail[:, :], lv_ps2[:, :], ACT.Tanh,
                                     bias=w_sb_b[:, 1:2])
                lv_ps3 = psr.tile([1, BL], F32, tag="psmall")
                diff = sbr.tile([1, BL], F32, tag="diff")
                nc.tensor.matmul(lv_ps3[:, :], w_diff[:, :], qvT[:, :],
                                 start=True, stop=True)
                nc.scalar.activation(diff[:, :], lv_ps3[:, :], ACT.Tanh,
                                     bias=w_sb_b[:, 2:3])

                # global success/failure counts (use FULL inputs, same all cores)
                sigs = sbr.tile([1, 2], F32, tag="sigs")
                for ci, full in enumerate([right_full, wrong_full]):
                    fin = sbr.tile([B, S], I32, tag="fin")
                    nc.sync.dma_start(out=fin[:, :], in_=full[:, :])
                    ff = sbr.tile([B, S], F32, tag="ff")
                    nc.vector.tensor_copy(ff[:, :], fin[:, :])
                    fc = sbr.tile([B, S], F32, tag="fc")
                    nc.vector.tensor_scalar(fc[:, :], ff[:, :], 1.0, None, ALU.min)
                    cs = sbr.tile([B, 1], F32, tag="cs")
                    nc.vector.tensor_reduce(cs[:, :], fc[:, :], AX.X, ALU.add)
                    cnt_ps = psr.tile([1, 1], F32, tag="psmall")
                    nc.tensor.matmul(cnt_ps[:, :], cs[:, :], w_ones_c32[:, :],
                                     start=True, stop=True)
                    nc.scalar.activation(sigs[:, ci:ci + 1], cnt_ps[:, :],
                                         ACT.Sigmoid)

                # out = succ*sig(sc) + fail*sig(fc) - 2*diff
                t1 = sbr.tile([1, BL], F32, tag="t1")
                nc.vector.tensor_scalar_mul(t1[:, :], succ[:, :], sigs[:, 0:1])
                t2 = sbr.tile([1, BL], F32, tag="t2")
                nc.vector.tensor_scalar_mul(t2[:, :], fail[:, :], sigs[:, 1:2])
                t3 = sbr.tile([1, BL], F32, tag="t3")
                nc.vector.tensor_scalar_mul(t3[:, :], diff[:, :], -2.0)
                o1 = sbr.tile([1, BL], F32, tag="o1")
                nc.vector.tensor_tensor(o1[:, :], t1[:, :], t2[:, :], ALU.add)
                o2 = sbr.tile([1, BL], F32, tag="o2")
                nc.vector.tensor_tensor(o2[:, :], o1[:, :], t3[:, :], ALU.add)
                nc.sync.dma_start(out=out_d[:, :], in_=o2[:, :])

                if DEBUG:
                    nc.sync.dma_start(out=dbg_m[:, :], in_=m_sb[:, :])
                    nc.sync.dma_start(out=dbg_rr[:, :], in_=rr[:, :])

    nc.compile()
    return nc


_PROGRAM = None


def _get_program():
    global _PROGRAM
    if _PROGRAM is None:
        _PROGRAM = _build_program()
    return _PROGRAM


def _host_inputs(inputs):
    """Build the per-core in_maps from the full problem inputs."""
    f16 = np.float16
    f32 = np.float32
    ri = np.asarray(inputs["right_input"]).astype(np.int64)
    wi = np.asarray(inputs["wrong_input"]).astype(np.int64)
    tg = np.asarray(inputs["target_id"]).astype(np.int64)
    q_emb = np.asarray(inputs["q_emb"], dtype=f32)
    i_emb = np.asarray(inputs["i_emb"], dtype=f32)

    def W(name):
        return np.asarray(inputs[name], dtype=f32)

    common = {
        "i_emb": i_emb,
        "q_emb": q_emb,
        "erase_Wt": np.ascontiguousarray(W("erase_W").T).astype(f16),
        "add_Wt": np.ascontiguousarray(W("add_W").T).astype(f16),
        "key_Wt": np.ascontiguousarray(W("key_W").T).astype(f16),
        "erase_b_row": W("erase_b").reshape(1, -1).astype(f16),
        "add_b_row": W("add_b").reshape(1, -1).astype(f16),
        "rsum_Wt0": np.ascontiguousarray(W("rsum_W")[:, :VD].T).astype(f16),
        "rsum_Wt1": np.ascontiguousarray(W("rsum_W")[:, VD:].T).astype(f16),
        "wsum_Wt0": np.ascontiguousarray(W("wsum_W")[:, :VD].T).astype(f16),
        "wsum_Wt1": np.ascontiguousarray(W("wsum_W")[:, VD:].T).astype(f16),
        "rsum_b_col": W("rsum_b").reshape(-1, 1).astype(f32),
        "wsum_b_col": W("wsum_b").reshape(-1, 1).astype(f32),
        "succ_Wt": np.ascontiguousarray(W("succ_W").T).astype(f16),
        "fail_Wt": np.ascontiguousarray(W("fail_W").T).astype(f16),
        "diff_Wt": np.ascontiguousarray(W("diff_W").T).astype(f16),
        "succ_b": W("succ_b").reshape(1, 1).astype(f32),
        "fail_b": W("fail_b").reshape(1, 1).astype(f32),
        "diff_b": W("diff_b").reshape(1, 1).astype(f32),
        "rmem0": W("right_mem_init").astype(f16),
        "wmem0": W("wrong_mem_init").astype(f16),
        "ones_row": np.ones((1, 128), dtype=f16),
        "ones_col32": np.ones((128, 1), dtype=f32),
        "id128": np.eye(128, dtype=f16),
        "right_full": ri.astype(np.int32),
        "wrong_full": wi.astype(np.int32),
    }

    in_maps = []
    for core in range(NCORE):
        rows = slice(core * BL, (core + 1) * BL)
        # inter ids per (t, r): r<BL -> right, else wrong
        inter = np.empty((S, NR), dtype=np.int64)
        inter[:, :BL] = ri[rows].T
        inter[:, BL:] = wi[rows].T
        qid = inter - Q * (inter > Q)
        # i-table gathers, pair layout: pair p rows [t1 | t2 | t1]
        idx_ip = np.empty((KF, NP), dtype=np.int32)
        for p in range(NP):
            idx_ip[0:NR, p] = inter[2 * p]
            idx_ip[NR:2 * NR, p] = inter[2 * p + 1]
            idx_ip[2 * NR:KF, p] = inter[2 * p]
        # q-table gathers, step layout (128-row chunks of t-major rows)
        flat_q = qid.reshape(-1)
        idx_q = flat_q.reshape(NGQ, 128).T.astype(np.int32)
        idx_t = tg[rows].reshape(BL, 1).astype(np.int32)
        in_maps.append({**common, "idx_i": np.ascontiguousarray(idx_ip),
                        "idx_q": np.ascontiguousarray(idx_q),
                        "idx_t": idx_t})
    return in_maps


def run_spmd(inputs, trace=False):
    nc = _get_program()
    in_maps = _host_inputs(inputs)
    res = run_bass_kernel_spmd(nc, in_maps, core_ids=list(range(NCORE)),
                               trace=trace)
    out = np.concatenate([res.results[i]["out"] for i in range(NCORE)], axis=0)
    return out.astype(np.float32), res


def kernel(**inputs):
    out, _ = run_spmd(inputs, trace=False)
    return out


# revision 14
# speedup vs baseline: 1.8965x; 1.4358x over previous
"""DiKT (DKVMN-style knowledge tracing) Trainium2 kernel.

Self-contained: builds a Bass/Tile program, shards batch over 8 NeuronCores
(pure data parallel, 16 batch rows per core), runs via run_bass_kernel_spmd.

Algorithm per core (B_loc=16, V=128, C=64, S=128 steps):
  Both value memories (right/wrong) live as ONE SBUF tensor m[v=128, col=2048]
  with col = r*64 + c, r = mem*16 + b.  Per step:
     m' = m * (1 - e x w) + a x w

Two consecutive steps are FUSED into one update (64 pairs):
     Shat = S1*S2 = 1 - e1 x w1 - e2 x w2 + (e1e2) x (w1w2)
     Ahat = S2*A1 + A2 = a1 x w1 - (a1e2) x (w1w2) + a2 x w2
Both are rank-3 sums of outer products, built by ONE K=96 TensorE matmul
against a 3-group block-diagonal rhs (per group g, row r: the (r,c) diagonal
block carries [-w1 | -w2 | +w1w2]).  lhsT groups: [e1 | e2 | e1e2] for Shat,
[-a1 | -a2 | -a1e2] for Ahat (signs make every product come out right).

Consumption per pair (m' = m*(1+Shat') + Ahat, Shat' = Shat-1 from PE):
  right memory (cols 0:1024):  ACT evacuates 1+Shat' and Ahat to fp16 SBUF,
     DVE multiplies, GpSimd adds.
  wrong memory (cols 1024:2048): DVE scalar_tensor_tensor (+1, mult) straight
     from PSUM, DVE add straight from PSUM.

All per-step e, a, w are precomputed up front from embedding gathers
(indirect DMA) + small matmuls; the fused block-diagonal rhs for all pairs
is staged in DRAM and streamed during the loop.
"""

import numpy as np

import concourse.mybir as mybir
from concourse import bass, bacc, tile
from concourse.bass_utils import run_bass_kernel_spmd

F16 = mybir.dt.float16
F32 = mybir.dt.float32
I32 = mybir.dt.int32
ALU = mybir.AluOpType
ACT = mybir.ActivationFunctionType
AX = mybir.AxisListType

# model dims
KD = 128      # KEY_DIM
VD = 128      # VALUE_DIM
SD = 128      # SUMMARY_DIM
Q = 10000     # QUESTION_NUM
C = 64        # CONCEPT_NUM
B = 128       # full batch
S = 128       # seq len
NCORE = 8
BL = B // NCORE          # 16 batch rows per core
NR = 2 * BL              # 32 rows per step (right+wrong)
COLS = NR * C            # 2048 memory columns per core
NP = S // 2              # 64 step pairs
KF = 3 * NR              # 96 contraction rows per fused matmul
PB = KF * COLS           # 196608 elements per pair block in wd_dram
NGQ = (S * NR) // 128    # 32 q-side gather chunks of 128 rows

DEBUG = False


def _build_program():
    nc = bacc.Bacc(trn_type="TRN2", target_bir_lowering=False, num_devices=NCORE,
                   num_swdge_queues=4)

    # ---- DRAM inputs ----
    i_emb = nc.dram_tensor("i_emb", [2 * Q + 1, VD], F32, kind="ExternalInput")
    q_emb = nc.dram_tensor("q_emb", [Q + 1, KD], F32, kind="ExternalInput")
    idx_i = nc.dram_tensor("idx_i", [128, NGQ], I32, kind="ExternalInput")
    idx_q = nc.dram_tensor("idx_q", [128, NGQ], I32, kind="ExternalInput")
    idx_t = nc.dram_tensor("idx_t", [BL, 1], I32, kind="ExternalInput")

    erase_Wt = nc.dram_tensor("erase_Wt", [VD, VD], F16, kind="ExternalInput")
    add_Wt = nc.dram_tensor("add_Wt", [VD, VD], F16, kind="ExternalInput")
    key_Wt = nc.dram_tensor("key_Wt", [KD, C], F16, kind="ExternalInput")
    erase_b_row = nc.dram_tensor("erase_b_row", [1, 4 * VD], F16, kind="ExternalInput")
    add_b_row = nc.dram_tensor("add_b_row", [1, 4 * VD], F16, kind="ExternalInput")
    rsum_Wt0 = nc.dram_tensor("rsum_Wt0", [VD, SD], F16, kind="ExternalInput")
    rsum_Wt1 = nc.dram_tensor("rsum_Wt1", [KD, SD], F16, kind="ExternalInput")
    wsum_Wt0 = nc.dram_tensor("wsum_Wt0", [VD, SD], F16, kind="ExternalInput")
    wsum_Wt1 = nc.dram_tensor("wsum_Wt1", [KD, SD], F16, kind="ExternalInput")
    rsum_b_col = nc.dram_tensor("rsum_b_col", [SD, 1], F32, kind="ExternalInput")
    wsum_b_col = nc.dram_tensor("wsum_b_col", [SD, 1], F32, kind="ExternalInput")
    succ_Wt = nc.dram_tensor("succ_Wt", [SD, 1], F16, kind="ExternalInput")
    fail_Wt = nc.dram_tensor("fail_Wt", [SD, 1], F16, kind="ExternalInput")
    diff_Wt = nc.dram_tensor("diff_Wt", [KD, 1], F16, kind="ExternalInput")
    succ_b = nc.dram_tensor("succ_b", [1, 1], F32, kind="ExternalInput")
    fail_b = nc.dram_tensor("fail_b", [1, 1], F32, kind="ExternalInput")
    diff_b = nc.dram_tensor("diff_b", [1, 1], F32, kind="ExternalInput")
    rmem0 = nc.dram_tensor("rmem0", [VD, C], F16, kind="ExternalInput")
    wmem0 = nc.dram_tensor("wmem0", [VD, C], F16, kind="ExternalInput")
    ones_row = nc.dram_tensor("ones_row", [1, 128], F16, kind="ExternalInput")
    ones_col32 = nc.dram_tensor("ones_col32", [128, 1], F32, kind="ExternalInput")
    id128 = nc.dram_tensor("id128", [128, 128], F16, kind="ExternalInput")
    right_full = nc.dram_tensor("right_full", [B, S], I32, kind="ExternalInput")
    wrong_full = nc.dram_tensor("wrong_full", [B, S], I32, kind="ExternalInput")

    out_d = nc.dram_tensor("out", [BL, 1], F32, kind="ExternalOutput")
    if DEBUG:
        dbg_m = nc.dram_tensor("dbg_m", [VD, COLS], F16, kind="ExternalOutput")
        dbg_rr = nc.dram_tensor("dbg_rr", [VD, NR], F32, kind="ExternalOutput")

    # fused block-diagonal rhs for every pair, flat fp16:
    # pair p, group g (0..2), row r (0..31): diag block at
    #   p*PB + g*65536 + r*2112, 64 wide
    wd_dram = nc.dram_tensor("wd_dram", [NP * PB], F16)

    # ---- persistent SBUF ----
    sb = lambda name, shape, dt: nc.alloc_sbuf_tensor(name, shape, dt)
    m_sb = sb("m_sb", [VD, COLS], F16)
    vecT = sb("vecT", [128, NP * 64], F16)   # i_emb rows [t1|t2] per pair, transposed
    qT = sb("qT", [128, NGQ * 128], F16)
    eP = sb("eP", [KF, NP * 128], F16)       # [e1 | e2 | e1e2] per pair
    naP = sb("naP", [KF, NP * 128], F16)     # [-a1 | -a2 | -a1e2] per pair
    scrB = sb("scrB", [KF, NP * 128], F16)   # e2 staging at group-2 partitions
    w_all = sb("w_all", [128, NGQ * C], F16)  # -w per step (4 steps/chunk)
    wstB = sb("wstB", [KF, NGQ * C], F16)    # -w2 at w1 partitions
    wprodB = sb("wprodB", [KF, NGQ * C], F16)  # w1*w2
    w_eWt = sb("w_eWt", [VD, VD], F16)
    w_aWt = sb("w_aWt", [VD, VD], F16)
    w_kWt = sb("w_kWt", [KD, C], F16)
    w_eb = sb("w_eb", [1, 4 * VD], F16)
    w_ab = sb("w_ab", [1, 4 * VD], F16)
    w_ones = sb("w_ones", [1, 128], F16)
    w_ones_c32 = sb("w_ones_c32", [128, 1], F32)
    w_id = sb("w_id", [128, 128], F16)
    idx_i_sb = sb("idx_i_sb", [128, NGQ], I32)
    idx_q_sb = sb("idx_q_sb", [128, NGQ], I32)
    idx_t_sb = sb("idx_t_sb", [BL, 1], I32)
    w_rs0 = sb("w_rs0", [VD, SD], F16)
    w_rs1 = sb("w_rs1", [KD, SD], F16)
    w_ws0 = sb("w_ws0", [VD, SD], F16)
    w_ws1 = sb("w_ws1", [KD, SD], F16)
    w_rsb = sb("w_rsb", [SD, 1], F32)
    w_wsb = sb("w_wsb", [SD, 1], F32)
    w_succ = sb("w_succ", [SD, 1], F16)
    w_fail = sb("w_fail", [SD, 1], F16)
    w_diff = sb("w_diff", [KD, 1], F16)
    w_sb_b = sb("w_sb_b", [1, 3], F32)  # succ_b, fail_b, diff_b columns 0..2
    zeros2k = sb("zeros2k", [128, COLS], F16)

    with tile.TileContext(nc) as tc:
        with tc.tile_pool(name="sbp", bufs=3) as sbp:
            # ---------- load constants ----------
            for dst, src in [
                (w_eWt, erase_Wt), (w_aWt, add_Wt), (w_kWt, key_Wt),
                (w_eb, erase_b_row), (w_ab, add_b_row), (w_ones, ones_row),
                (w_ones_c32, ones_col32), (w_id, id128),
                (idx_i_sb, idx_i), (idx_q_sb, idx_q), (idx_t_sb, idx_t),
                (w_rs0, rsum_Wt0), (w_rs1, rsum_Wt1),
                (w_ws0, wsum_Wt0), (w_ws1, wsum_Wt1),
                (w_rsb, rsum_b_col), (w_wsb, wsum_b_col),
                (w_succ, succ_Wt), (w_fail, fail_Wt), (w_diff, diff_Wt),
            ]:
                nc.sync.dma_start(out=dst[:, :], in_=src[:, :])
            nc.sync.dma_start(out=w_sb_b[:, 0:1], in_=succ_b[:, :])
            nc.sync.dma_start(out=w_sb_b[:, 1:2], in_=fail_b[:, :])
            nc.sync.dma_start(out=w_sb_b[:, 2:3], in_=diff_b[:, :])

            # zero-fill wd_dram (24 MiB fp16) from a zeroed sbuf tile
            nc.gpsimd.memset(zeros2k[:, :], 0.0)
            for g in range(NP * PB // (128 * 2048)):
                nc.scalar.dma_start(
                    out=bass.AP(wd_dram, g * 128 * 2048, [[2048, 128], [1, 2048]]),
                    in_=zeros2k[:, :],
                )

            # init m: broadcast mem inits over the 16 batch blocks
            rmem_t = sbp.tile([VD, C], F16, tag="memi")
            nc.sync.dma_start(out=rmem_t[:, :], in_=rmem0[:, :])
            wmem_t = sbp.tile([VD, C], F16, tag="memi2")
            nc.sync.dma_start(out=wmem_t[:, :], in_=wmem0[:, :])
            for r in range(NR):
                srct = rmem_t if r < BL else wmem_t
                nc.vector.tensor_copy(m_sb[:, r * C:(r + 1) * C], srct[:, :])

            # ---------- gathers + transposes (t-major 128-row chunks) ----------
            with tc.tile_pool(name="pst", bufs=2, space="PSUM") as psp:
                for g in range(NGQ):
                    lo = g * 128
                    gi32 = sbp.tile([128, VD], F32, tag="gi32")
                    nc.gpsimd.indirect_dma_start(
                        out=gi32[:, :], out_offset=None,
                        in_=i_emb[:, :],
                        in_offset=bass.IndirectOffsetOnAxis(
                            ap=idx_i_sb[:, g:g + 1], axis=0),
                    )
                    gi16 = sbp.tile([128, VD], F16, tag="gi16")
                    nc.vector.tensor_copy(gi16[:, :], gi32[:, :])
                    tps = psp.tile([128, 128], F16, tag="tp")
                    nc.tensor.transpose(tps[:, :], gi16[:, :], w_id[:, :])
                    nc.vector.tensor_copy(vecT[:, lo:lo + 128], tps[:, :])
                for g in range(NGQ):
                    lo = g * 128
                    gq32 = sbp.tile([128, KD], F32, tag="gq32")
                    nc.gpsimd.indirect_dma_start(
                        out=gq32[:, :], out_offset=None,
                        in_=q_emb[:, :],
                        in_offset=bass.IndirectOffsetOnAxis(
                            ap=idx_q_sb[:, g:g + 1], axis=0),
                    )
                    gq16 = sbp.tile([128, KD], F16, tag="gq16")
                    nc.vector.tensor_copy(gq16[:, :], gq32[:, :])
                    tps2 = psp.tile([128, 128], F16, tag="tp2")
                    nc.tensor.transpose(tps2[:, :], gq16[:, :], w_id[:, :])
                    nc.vector.tensor_copy(qT[:, lo:lo + 128], tps2[:, :])

            # ---------- e/a precompute (pair layout) ----------
            # grouped by ACT table set: all Sigmoid, then all Tanh, then Exp
            with tc.tile_pool(name="psz", bufs=2, space="PSUM") as psp:
                # e/a matmuls: per pair p, lhsT cols [t1|t2] at vecT[:, 64p:64p+64];
                # group rows: [e(t1) 0:64 incl e(t2) | e(t1) again at 64:96 via
                # a second M=32 matmul into col-group 64].  Bias via one K=1
                # matmul per 4-pair batch (repeated-bias rhs).
                for pg in range(NP // 4):
                    eps = psp.tile([KF, 512], F32, tag="eps")
                    for k in range(4):
                        p = 4 * pg + k
                        vl = vecT[:, 64 * p:64 * (p + 1)]
                        nc.tensor.matmul(eps[0:64, 128 * k:128 * (k + 1)],
                                         vl, w_eWt[:, :],
                                         start=True, stop=True)
                        nc.tensor.matmul(eps[64:96, 128 * k:128 * (k + 1)],
                                         vecT[:, 64 * p:64 * p + 32], w_eWt[:, :],
                                         start=True, stop=True,
                                         tile_position=(0, 64))
                    nc.tensor.matmul(eps[:, :], w_ones[:, :KF], w_eb[:, :],
                                     start=False, stop=True, skip_group_check=True)
                    nc.scalar.activation(eP[:, 512 * pg:512 * (pg + 1)],
                                         eps[:, :], ACT.Sigmoid)
                for pg in range(NP // 4):
                    aps = psp.tile([KF, 512], F32, tag="aps")
                    for k in range(4):
                        p = 4 * pg + k
                        vl = vecT[:, 64 * p:64 * (p + 1)]
                        nc.tensor.matmul(aps[0:64, 128 * k:128 * (k + 1)],
                                         vl, w_aWt[:, :],
                                         start=True, stop=True)
                        nc.tensor.matmul(aps[64:96, 128 * k:128 * (k + 1)],
                                         vecT[:, 64 * p:64 * p + 32], w_aWt[:, :],
                                         start=True, stop=True,
                                         tile_position=(0, 64))
                    nc.tensor.matmul(aps[:, :], w_ones[:, :KF], w_ab[:, :],
                                     start=False, stop=True, skip_group_check=True)
                    nc.scalar.activation(naP[:, 512 * pg:512 * (pg + 1)],
                                         aps[:, :], ACT.Tanh, scale=-1.0)
                # group-2 products, batched over all pairs: copy e2 rows to the
                # group-2 partitions once, then two wide in-place multiplies
                for q4 in range(4):
                    cl = q4 * NP * 32
                    nc.sync.dma_start(out=scrB[64:96, cl:cl + NP * 32],
                                      in_=eP[32:64, cl:cl + NP * 32])
                for q4 in range(4):
                    cl = q4 * NP * 32
                    nc.vector.tensor_tensor(naP[64:96, cl:cl + NP * 32],
                                            naP[64:96, cl:cl + NP * 32],
                                            scrB[64:96, cl:cl + NP * 32], ALU.mult)
                    nc.vector.tensor_tensor(eP[64:96, cl:cl + NP * 32],
                                            eP[64:96, cl:cl + NP * 32],
                                            scrB[64:96, cl:cl + NP * 32], ALU.mult)

                # w = softmax(qv @ key_W.T) (negated) into w_all
                for g in range(NGQ):
                    lo = g * 128
                    zps = psp.tile([128, C], F32, tag="zps")
                    nc.tensor.matmul(zps[:, :], qT[:, lo:lo + 128], w_kWt[:, :],
                                     start=True, stop=True)
                    wex = sbp.tile([128, C], F32, tag="wex")
                    nc.scalar.activation(wex[:, :], zps[:, :], ACT.Exp)
                    sm = sbp.tile([128, 1], F32, tag="sm")
                    nc.vector.tensor_reduce(sm[:, :], wex[:, :], AX.X, ALU.add)
                    rc = sbp.tile([128, 1], F32, tag="rc")
                    nc.vector.reciprocal(rc[:, :], sm[:, :])
                    nc.vector.tensor_scalar(w_all[:, g * C:(g + 1) * C],
                                            wex[:, :], rc[:, :], -1.0,
                                            ALU.mult, ALU.mult)
                # batched pair products: w2 rows to w1 partitions, multiply
                for r0 in (0, 64):
                    nc.sync.dma_start(out=wstB[r0:r0 + 32, :],
                                      in_=w_all[r0 + 32:r0 + 64, :])
                for r0 in (0, 64):
                    nc.vector.tensor_tensor(wprodB[r0:r0 + 32, :],
                                            w_all[r0:r0 + 32, :],
                                            wstB[r0:r0 + 32, :], ALU.mult)
                # batched diagonal scatters: 2 parities x 3 groups
                # even pairs p=2g at partitions [0:64), odd p=2g+1 at [64:128)
                for parity in range(2):
                    pr = 64 * parity
                    for gi, src_t, srow in (
                        (0, w_all, pr), (1, w_all, pr + 32), (2, wprodB, pr)):
                        nc.sync.dma_start(
                            out=bass.AP(wd_dram, parity * PB + gi * 65536,
                                        [[COLS + C, NR], [2 * PB, NGQ], [1, C]]),
                            in_=src_t[srow:srow + 32, :].rearrange(
                                "p (g c) -> p g c", c=C),
                        )

            # ---------- the fused pair recurrence ----------
            H = COLS // 2
            with tc.tile_pool(name="psl", bufs=1, space="PSUM") as psl, \
                 tc.tile_pool(name="wdp", bufs=2) as wdp, \
                 tc.tile_pool(name="sul", bufs=2) as sul:
                for p in range(NP):
                    wd4 = wdp.tile([KF, COLS], F16, tag="wd4")
                    nc.sync.dma_start(
                        out=wd4[:, :],
                        in_=bass.AP(wd_dram, p * PB, [[2048, KF], [1, 2048]]),
                    )
                    eL = eP[:, p * 128:(p + 1) * 128]
                    aL = naP[:, p * 128:(p + 1) * 128]
                    Eh0 = psl.tile([128, H], F32, tag="Eh0")
                    Ah0 = psl.tile([128, H], F32, tag="Ah0")
                    Eh1 = psl.tile([128, H], F32, tag="Eh1")
                    Ah1 = psl.tile([128, H], F32, tag="Ah1")
                    for hk in range(2):
                        c0, c1 = 512 * hk, 512 * (hk + 1)
                        nc.tensor.matmul(Eh1[:, c0:c1], eL, wd4[:, H + c0:H + c1],
                                         start=True, stop=True)
                        nc.tensor.matmul(Ah1[:, c0:c1], aL, wd4[:, H + c0:H + c1],
                                         start=True, stop=True)
                    for hk in range(2):
                        c0, c1 = 512 * hk, 512 * (hk + 1)
                        nc.tensor.matmul(Eh0[:, c0:c1], eL, wd4[:, c0:c1],
                                         start=True, stop=True)
                        nc.tensor.matmul(Ah0[:, c0:c1], aL, wd4[:, c0:c1],
                                         start=True, stop=True)
                    # wrong memory: PSUM-direct on DVE
                    u1 = sul.tile([128, H], F16, tag="u1")
                    nc.vector.scalar_tensor_tensor(u1[:, :], Eh1[:, :], 1.0,
                                                   m_sb[:, H:COLS],
                                                   ALU.add, ALU.mult)
                    nc.vector.tensor_tensor(m_sb[:, H:COLS], u1[:, :], Ah1[:, :],
                                            ALU.add)
                    # right memory: ACT-evacuated, DVE mul, POOL add
                    S0 = sul.tile([128, H], F16, tag="S0")
                    nc.scalar.activation(S0[:, :], Eh0[:, :], ACT.Identity,
                                         bias=1.0)
                    A0 = sul.tile([128, H], F16, tag="A0")
                    nc.scalar.activation(A0[:, :], Ah0[:, :], ACT.Copy, bias=0.0)
                    u0 = sul.tile([128, H], F16, tag="u0")
                    nc.vector.tensor_tensor(u0[:, :], m_sb[:, 0:H], S0[:, :],
                                            ALU.mult)
                    nc.gpsimd.tensor_tensor(m_sb[:, 0:H], u0[:, :], A0[:, :],
                                            ALU.add)

            # ---------- readout + head ----------
            with tc.tile_pool(name="psr", bufs=1, space="PSUM") as psr, \
                 tc.tile_pool(name="sbr", bufs=1) as sbr:
                # target question embedding, transposed
                qv32 = sbr.tile([BL, KD], F32, tag="qv32")
                nc.gpsimd.indirect_dma_start(
                    out=qv32[:, :], out_offset=None,
                    in_=q_emb[:, :],
                    in_offset=bass.IndirectOffsetOnAxis(ap=idx_t_sb[:, 0:1], axis=0),
                )
                qv16 = sbr.tile([BL, KD], F16, tag="qv16")
                nc.vector.tensor_copy(qv16[:, :], qv32[:, :])
                qvT_ps = psr.tile([KD, BL], F16, tag="psmall")
                nc.tensor.transpose(qvT_ps[:, :], qv16[:, :], w_id[:BL, :BL])
                qvT = sbr.tile([KD, BL], F16, tag="qvT")
                nc.vector.tensor_copy(qvT[:, :], qvT_ps[:, :])

                # wt = softmax(qv @ key_W.T)
                zt = psr.tile([BL, C], F32, tag="psmall")
                nc.tensor.matmul(zt[:, :], qvT[:, :], w_kWt[:, :], start=True,
                                 stop=True)
                wext = sbr.tile([BL, C], F32, tag="wext")
                nc.scalar.activation(wext[:, :], zt[:, :], ACT.Exp)
                smt = sbr.tile([BL, 1], F32, tag="smt")
                nc.vector.tensor_reduce(smt[:, :], wext[:, :], AX.X, ALU.add)
                rct = sbr.tile([BL, 1], F32, tag="rct")
                nc.vector.reciprocal(rct[:, :], smt[:, :])
                wt16 = sbr.tile([BL, C], F16, tag="wt16")
                nc.vector.tensor_scalar_mul(wt16[:, :], wext[:, :], rct[:, :])
                # flatten to (1, 2048): [right blocks | wrong blocks], both = wt
                wtf = sbr.tile([1, COLS], F16, tag="wtf")
                nc.gpsimd.dma_start(out=wtf[0:1, 0:BL * C], in_=wt16[:, :])
                nc.gpsimd.dma_start(out=wtf[0:1, BL * C:COLS], in_=wt16[:, :])
                # broadcast over partitions via K=1 matmul
                wb_ps = psr.tile([128, COLS], F32, tag="wb_ps")
                for k in range(4):
                    nc.tensor.matmul(wb_ps[:, 512 * k:512 * (k + 1)], w_ones[:, :],
                                     wtf[:, 512 * k:512 * (k + 1)],
                                     start=True, stop=True)
                wb = sbr.tile([128, COLS], F16, tag="wb")
                nc.scalar.activation(wb[:, :], wb_ps[:, :], ACT.Copy, bias=0.0)
                u2 = sbr.tile([128, COLS], F16, tag="u2")
                nc.vector.tensor_tensor(u2[:, :], m_sb[:, :], wb[:, :], ALU.mult)
                rr = sbr.tile([VD, NR], F32, tag="rr")
                nc.vector.tensor_reduce(
                    rr[:, :], u2[:].rearrange("p (r c) -> p r c", c=C), AX.X,
                    ALU.add)
                rr16 = sbr.tile([VD, NR], F16, tag="rr16")
                nc.vector.tensor_copy(rr16[:, :], rr[:, :])

                # r_sum / w_sum: (SD, BL)
                rs_ps = psr.tile([SD, BL], F32, tag="psmall")
                nc.tensor.matmul(rs_ps[:, :], w_rs0[:, :], rr16[:, 0:BL],
                                 start=True, stop=False)
                nc.tensor.matmul(rs_ps[:, :], w_rs1[:, :], qvT[:, :],
                                 start=False, stop=True)
                rsum = sbr.tile([SD, BL], F16, tag="rsum")
                nc.scalar.activation(rsum[:, :], rs_ps[:, :], ACT.Tanh,
                                     bias=w_rsb[:, :])
                ws_ps = psr.tile([SD, BL], F32, tag="psmall")
                nc.tensor.matmul(ws_ps[:, :], w_ws0[:, :], rr16[:, BL:NR],
                                 start=True, stop=False)
                nc.tensor.matmul(ws_ps[:, :], w_ws1[:, :], qvT[:, :],
                                 start=False, stop=True)
                wsum = sbr.tile([SD, BL], F16, tag="wsum")
                nc.scalar.activation(wsum[:, :], ws_ps[:, :], ACT.Tanh,
                                     bias=w_wsb[:, :])

                # success/failure/difficulty levels: (1, BL)
                lv_ps = psr.tile([1, BL], F32, tag="psmall")
                succ = sbr.tile([1, BL], F32, tag="succ")
                nc.tensor.matmul(lv_ps[:, :], w_succ[:, :], rsum[:, :],
                                 start=True, stop=True)
                nc.scalar.activation(succ[:, :], lv_ps[:, :], ACT.Tanh,
                                     bias=w_sb_b[:, 0:1])
                lv_ps2 = psr.tile([1, BL], F32, tag="psmall")
                fail = sbr.tile([1, BL], F32, tag="fail")
                nc.tensor.matmul(lv_ps2[:, :], w_fail[:, :], wsum[:, :],
                                 start=True, stop=True)
                nc.scalar.activation(fail[:, :], lv_ps2[:, :], ACT.Tanh,
                                     bias=w_sb_b[:, 1:2])
                lv_ps3 = psr.tile([1, BL], F32, tag="psmall")
                diff = sbr.tile([1, BL], F32, tag="diff")
                nc.tensor.matmul(lv_ps3[:, :], w_diff[:, :], qvT[:, :],
                                 start=True, stop=True)
                nc.scalar.activation(diff[:, :], lv_ps3[:, :], ACT.Tanh,
                                     bias=w_sb_b[:, 2:3])

                # global success/failure counts (use FULL inputs, same all cores)
                sigs = sbr.tile([1, 2], F32, tag="sigs")
                for ci, full in enumerate([right_full, wrong_full]):
                    fin = sbr.tile([B, S], I32, tag="fin")
                    nc.sync.dma_start(out=fin[:, :], in_=full[:, :])
                    ff = sbr.tile([B, S], F32, tag="ff")
                    nc.vector.tensor_copy(ff[:, :], fin[:, :])
                    fc = sbr.tile([B, S], F32, tag="fc")
                    nc.vector.tensor_scalar(fc[:, :], ff[:, :], 1.0, None, ALU.min)
                    cs = sbr.tile([B, 1], F32, tag="cs")
                    nc.vector.tensor_reduce(cs[:, :], fc[:, :], AX.X, ALU.add)
                    cnt_ps = psr.tile([1, 1], F32, tag="psmall")
                    nc.tensor.matmul(cnt_ps[:, :], cs[:, :], w_ones_c32[:, :],
                                     start=True, stop=True)
                    nc.scalar.activation(sigs[:, ci:ci + 1], cnt_ps[:, :],
                                         ACT.Sigmoid)

                # out = succ*sig(sc) + fail*sig(fc) - 2*diff
                t1 = sbr.tile([1, BL], F32, tag="t1")
                nc.vector.tensor_scalar_mul(t1[:, :], succ[:, :], sigs[:, 0:1])
                t2 = sbr.tile([1, BL], F32, tag="t2")
                nc.vector.tensor_scalar_mul(t2[:, :], fail[:, :], sigs[:, 1:2])
                t3 = sbr.tile([1, BL], F32, tag="t3")
                nc.vector.tensor_scalar_mul(t3[:, :], diff[:, :], -2.0)
                o1 = sbr.tile([1, BL], F32, tag="o1")
                nc.vector.tensor_tensor(o1[:, :], t1[:, :], t2[:, :], ALU.add)
                o2 = sbr.tile([1, BL], F32, tag="o2")
                nc.vector.tensor_tensor(o2[:, :], o1[:, :], t3[:, :], ALU.add)
                nc.sync.dma_start(out=out_d[:, :], in_=o2[:, :])

                if DEBUG:
                    nc.sync.dma_start(out=dbg_m[:, :], in_=m_sb[:, :])
                    nc.sync.dma_start(out=dbg_rr[:, :], in_=rr[:, :])

    nc.compile()
    return nc


_PROGRAM = None


def _get_program():
    global _PROGRAM
    if _PROGRAM is None:
        _PROGRAM = _build_program()
    return _PROGRAM


def _host_inputs(inputs):
    """Build the per-core in_maps from the full problem inputs."""
    f16 = np.float16
    f32 = np.float32
    ri = np.asarray(inputs["right_input"]).astype(np.int64)
    wi = np.asarray(inputs["wrong_input"]).astype(np.int64)
    tg = np.asarray(inputs["target_id"]).astype(np.int64)
    q_emb = np.asarray(inputs["q_emb"], dtype=f32)
    i_emb = np.asarray(inputs["i_emb"], dtype=f32)

    def W(name):
        return np.asarray(inputs[name], dtype=f32)

    common = {
        "i_emb": i_emb,
        "q_emb": q_emb,
        "erase_Wt": np.ascontiguousarray(W("erase_W").T).astype(f16),
        "add_Wt": np.ascontiguousarray(W("add_W").T).astype(f16),
        "key_Wt": np.ascontiguousarray(W("key_W").T).astype(f16),
        "erase_b_row": np.tile(W("erase_b").reshape(1, -1), (1, 4)).astype(f16),
        "add_b_row": np.tile(W("add_b").reshape(1, -1), (1, 4)).astype(f16),
        "rsum_Wt0": np.ascontiguousarray(W("rsum_W")[:, :VD].T).astype(f16),
        "rsum_Wt1": np.ascontiguousarray(W("rsum_W")[:, VD:].T).astype(f16),
        "wsum_Wt0": np.ascontiguousarray(W("wsum_W")[:, :VD].T).astype(f16),
        "wsum_Wt1": np.ascontiguousarray(W("wsum_W")[:, VD:].T).astype(f16),
        "rsum_b_col": W("rsum_b").reshape(-1, 1).astype(f32),
        "wsum_b_col": W("wsum_b").reshape(-1, 1).astype(f32),
        "succ_Wt": np.ascontiguousarray(W("succ_W").T).astype(f16),
        "fail_Wt": np.ascontiguousarray(W("fail_W").T).astype(f16),
        "diff_Wt": np.ascontiguousarray(W("diff_W").T).astype(f16),
        "succ_b": W("succ_b").reshape(1, 1).astype(f32),
        "fail_b": W("fail_b").reshape(1, 1).astype(f32),
        "diff_b": W("diff_b").reshape(1, 1).astype(f32),
        "rmem0": W("right_mem_init").astype(f16),
        "wmem0": W("wrong_mem_init").astype(f16),
        "ones_row": np.ones((1, 128), dtype=f16),
        "ones_col32": np.ones((128, 1), dtype=f32),
        "id128": np.eye(128, dtype=f16),
        "right_full": ri.astype(np.int32),
        "wrong_full": wi.astype(np.int32),
    }

    in_maps = []
    for core in range(NCORE):
        rows = slice(core * BL, (core + 1) * BL)
        # inter ids per (t, r): r<BL -> right, else wrong
        inter = np.empty((S, NR), dtype=np.int64)
        inter[:, :BL] = ri[rows].T
        inter[:, BL:] = wi[rows].T
        qid = inter - Q * (inter > Q)
        # both tables: t-major 128-row chunks (4 steps x 32 rows per chunk)
        idx_ip = inter.reshape(-1).reshape(NGQ, 128).T.astype(np.int32)
        flat_q = qid.reshape(-1)
        idx_q = flat_q.reshape(NGQ, 128).T.astype(np.int32)
        idx_t = tg[rows].reshape(BL, 1).astype(np.int32)
        in_maps.append({**common, "idx_i": np.ascontiguousarray(idx_ip),
                        "idx_q": np.ascontiguousarray(idx_q),
                        "idx_t": idx_t})
    return in_maps


def run_spmd(inputs, trace=False):
    nc = _get_program()
    in_maps = _host_inputs(inputs)
    res = run_bass_kernel_spmd(nc, in_maps, core_ids=list(range(NCORE)),
                               trace=trace)
    out = np.concatenate([res.results[i]["out"] for i in range(NCORE)], axis=0)
    return out.astype(np.float32), res


def kernel(**inputs):
    out, _ = run_spmd(inputs, trace=False)
    return out
